# revision 1
# baseline (speedup 1.0000x reference)
"""Trainium2 Bass kernel for the 2-layer CIN (compressed interaction network).

Reference computation (per batch element b, embedding channel d):
  z0[hf=h*40+f]  = x[b,h,d] * x[b,f,d]              (h,f in 0..39)
  y0[o]          = relu(sum_hf W0[o,hf,d] * z0[hf] + b0[o])   -> x1[b,o,d]
  z1[hf=h1*40+f] = x1[b,h1,d] * x[b,f,d]            (h1 in 0..63)
  y1[o]          = relu(sum_hf W1[o,hf,d] * z1[hf] + b1[o])   -> x2[b,o,d]
  out[b] = [sum_d x[b,:,d] | sum_d x1[b,:,d] | sum_d x2[b,:,d]]   (2048, 168)

Sharding: 4-way batch x 2-way embedding-channel split (8 cores). Each core
computes partial d-sums for its 512-row batch shard over its 16 d-channels;
the host adds the two d-halves (no device-side collectives). The d-split
halves the weight traffic per core and doubles the free dimension of every
PE matmul to 512 columns, halving real-matmul instruction count.

Device algorithm per core (bf16 compute, fp32 PSUM accumulate):
  - x arrives host-transposed as xT[f, d, b] (40, 16*512) bf16; replicated x3
    across partition groups -> XF (128, 8192): partition p holds f(p) = p%40
    (rows 120-127 carry unused but finite values).
  - K-tiles of 128 partitions covering 120 hf rows each: tile t covers hf in
    [120t, 120t+120): h(p) = 3t + p//40, f(p) = p%40 (h-major flatten);
    partitions 120-127 are zero-padded so the 128-column selection
    stationary is FWL-eligible.
  - The h-replicated factor XH_t is built by a selection matmul on the PE:
    XH = SEL_t^T @ xT (SEL_t[k,p] = 1 iff k == 3t + p//40) into PSUM fp32.
  - ScalarE copies/casts XH PSUM->SBUF bf16 (VectorE takes every 6th copy);
    VectorE multiplies XH*XF -> z^T tile (128, 1024) bf16 in 2x mode.
  - Real matmuls: lhsT = W-tile (128, 64) per d (host-pretransposed, d-major
    free layout), rhs = z^T d-slice (120, 512), accumulating over K-tiles.
    The two d's of a group target partitions 0-63 / 64-127 of one PSUM tile,
    i.e. disjoint PE column groups, so they can overlap in the array.
  - ScalarE applies relu+bias (per-partition bias AP) -> x1T slices bf16,
    which feed layer 2's selection matmuls; VectorE accumulates sum_d.
  - Epilogue: PE-transpose the (64, 512) accumulators -> (512, 128), DMA out.
  - The x part of the output (sum_d x) is computed on the host in fp32.
"""

import os
from contextlib import ExitStack

import numpy as np
import ml_dtypes

import concourse.bass as bass
import concourse.bacc as bacc
import concourse.tile as tile
from concourse import mybir
from concourse.bass_utils import run_bass_kernel_spmd
from concourse.masks import make_identity

BF16 = mybir.dt.bfloat16
FP32 = mybir.dt.float32
NPBF16 = ml_dtypes.bfloat16

B, F, D = 2048, 40, 32
O0, O1 = 64, 64
NCORES = 8
NB = 4                      # batch shards
ND = 2                      # d shards
BC = B // NB                # 512 batch rows per core
DC = D // ND                # 16 embedding channels per core
H0, H1 = F * F, O0 * F      # 1600, 2560 contraction rows
HS = 120                    # hf rows covered per K-tile (3 h-blocks x 40 f)
KT = 128                    # K-tile partition count (rows 120-127 zero-padded
                            # so the 128-col selection stationary can use FWL)
NT0 = (H0 + HS - 1) // HS   # 14 K-tiles, layer 1
NT1 = (H1 + HS - 1) // HS   # 22 K-tiles, layer 2
DPG = 2                     # d-channels per group (one PSUM pair-tile)
DG = DC // DPG              # 8 d-groups
NCOL = DPG * BC             # 1024 free columns per chunk (d-major, b-minor)
NMM = 512                   # max fp32-PSUM matmul free size


def _build_bass(reps=1):
    nc = bacc.Bacc()
    xt = nc.declare_dram_parameter("xt", [F, DC * BC], BF16, isOutput=False)
    w0t = nc.declare_dram_parameter("w0t", [KT, NT0 * DC * O0], BF16, isOutput=False)
    w1t = nc.declare_dram_parameter("w1t", [KT, NT1 * DC * O1], BF16, isOutput=False)
    sel0 = nc.declare_dram_parameter("sel0", [F, NT0 * KT], BF16, isOutput=False)
    sel1 = nc.declare_dram_parameter("sel1", [O0, NT1 * KT], BF16, isOutput=False)
    b0 = nc.declare_dram_parameter("b0", [O0, 1], FP32, isOutput=False)
    b1 = nc.declare_dram_parameter("b1", [O1, 1], FP32, isOutput=False)
    out = nc.declare_dram_parameter("out", [BC, O0 + O1], FP32, isOutput=True)

    with ExitStack() as ctx:
        tc = ctx.enter_context(tile.TileContext(nc))
        singles = ctx.enter_context(tc.tile_pool(name="singles", bufs=1))
        xh_ps = ctx.enter_context(tc.tile_pool(name="xh_ps", bufs=2, space="PSUM"))
        y_ps = ctx.enter_context(tc.tile_pool(name="y_ps", bufs=4, space="PSUM"))
        xh_sb = ctx.enter_context(tc.tile_pool(name="xh_sb", bufs=4))
        z_sb = ctx.enter_context(tc.tile_pool(name="z_sb", bufs=4))
        x2_sb = ctx.enter_context(tc.tile_pool(name="x2_sb", bufs=2))
        o_sb = ctx.enter_context(tc.tile_pool(name="o_sb", bufs=2))

        # ---- resident tensors ----
        # XF: x^T replicated x3 across partition groups: partition p = rep*40+f
        xf = singles.tile([KT, DC * BC], BF16)
        xt_ap = xt[:]
        rep_src = bass.AP(
            tensor=xt_ap.tensor,
            offset=xt_ap.offset,
            ap=[[0, 3], [DC * BC, F], [1, DC * BC]],
        )
        pad_src = bass.AP(
            tensor=xt_ap.tensor,
            offset=xt_ap.offset,
            ap=[[DC * BC, KT - 3 * F], [1, DC * BC]],
        )
        w0s = singles.tile([KT, NT0, DC * O0], BF16)
        w1s = singles.tile([KT, NT1, DC * O1], BF16)
        sel0s = singles.tile([F, NT0, KT], BF16)
        sel1s = singles.tile([O0, NT1, KT], BF16)
        b0s = singles.tile([O0, 1], FP32)
        b1s = singles.tile([O1, 1], FP32)

        def load_inputs():
            # small tensors first: the first selection matmul gates on sel0s
            # and xf; W tiles are consumed one K-tile at a time, so they can
            # land progressively (W1 last - only needed for layer 2).
            nc.gpsimd.dma_start(out=sel0s, in_=sel0[:])
            nc.gpsimd.dma_start(out=xf[0:3 * F, :], in_=rep_src)
            nc.gpsimd.dma_start(out=xf[3 * F:KT, :], in_=pad_src)
            nc.gpsimd.dma_start(out=sel1s, in_=sel1[:])
            nc.gpsimd.dma_start(out=b0s, in_=b0[:])
            nc.gpsimd.dma_start(out=b1s, in_=b1[:])
            # W pre-laid on host in SBUF order: one partition-scatter DMA each
            nc.sync.dma_start(out=w0s, in_=w0t[:])
            nc.gpsimd.dma_start(out=w1s, in_=w1t[:])

        ident = singles.tile([128, 128], FP32)
        make_identity(nc, ident)

        x1t = singles.tile([O0, DC * BC], BF16)   # x1^T, d-major free layout
        acc1 = singles.tile([O0, BC], FP32)
        acc2 = singles.tile([O1, BC], FP32)

        def layer(g, nt, sels, ws, rhs_src, kdim, odim):
            """One CIN layer for d-group g. Returns the (128, BC) PSUM pair."""
            col0 = g * NCOL
            # The two d's of the group share one (128, BC) PSUM tile: even d
            # at partitions 0-63, odd d at 64-127 -> disjoint PE column
            # groups, concurrent matmuls (tile_position inferred).
            yp = y_ps.tile([2 * odim, BC], FP32, tag="y", name=f"y_{g}")
            for t in range(nt):
                xh = xh_ps.tile([KT, NCOL], FP32, tag="xh")
                for h in range(NCOL // NMM):
                    nc.tensor.matmul(
                        xh[:, h * NMM:(h + 1) * NMM],
                        lhsT=sels[:, t, :],
                        rhs=rhs_src[0:kdim, col0 + h * NMM: col0 + (h + 1) * NMM],
                        start=True,
                        stop=True,
                    )
                z = z_sb.tile([KT, NCOL], BF16, tag="z")
                xhs = xh_sb.tile([KT, NCOL], BF16, tag="xhs")
                if t % 6 == 5:
                    nc.vector.tensor_copy(out=xhs, in_=xh)
                else:
                    nc.scalar.copy(out=xhs, in_=xh)
                nc.vector.tensor_mul(z, xhs, xf[:, col0:col0 + NCOL])
                for i in range(DPG):
                    d = g * DPG + i
                    nc.tensor.matmul(
                        yp[i * odim:(i + 1) * odim, :],
                        lhsT=ws[:, t, d * odim:(d + 1) * odim],
                        rhs=z[:, i * BC:(i + 1) * BC],
                        start=(t == 0),
                        stop=(t == nt - 1),
                    )
            return yp

        load_inputs()
        for rep in range(reps):
          nc.vector.memset(acc1, 0.0)
          nc.vector.memset(acc2, 0.0)
          for g in range(DG):
            col0 = g * NCOL
            yp0 = layer(g, NT0, sel0s, w0s, xf, F, O0)
            for i in range(DPG):
                nc.scalar.activation(
                    out=x1t[:, col0 + i * BC: col0 + (i + 1) * BC],
                    in_=yp0[i * O0:(i + 1) * O0, :],
                    func=mybir.ActivationFunctionType.Relu,
                    bias=b0s,
                    scale=1.0,
                )
                nc.vector.tensor_add(
                    acc1, acc1, x1t[:, col0 + i * BC: col0 + (i + 1) * BC]
                )
            yp1 = layer(g, NT1, sel1s, w1s, x1t, O0, O1)
            for i in range(DPG):
                x2 = x2_sb.tile([O1, BC], BF16, tag="x2")
                nc.scalar.activation(
                    out=x2,
                    in_=yp1[i * O1:(i + 1) * O1, :],
                    func=mybir.ActivationFunctionType.Relu,
                    bias=b1s,
                    scale=1.0,
                )
                nc.vector.tensor_add(acc2, acc2, x2)

          # ---- epilogue: transpose accumulators to (b, o) and store ----
          for bh in range(BC // 128):
            outT = o_sb.tile([128, O0 + O1], FP32, tag="outT")
            for acc, off in ((acc1, 0), (acc2, O0)):
                pt = y_ps.tile([128, 64], FP32, tag="y")
                nc.tensor.transpose(
                    pt, acc[:, bh * 128:(bh + 1) * 128], ident[0:64, 0:64]
                )
                nc.vector.tensor_copy(out=outT[:, off:off + 64], in_=pt)
            nc.sync.dma_start(
                out=out[bh * 128:(bh + 1) * 128, :], in_=outT
            )

    nc.compile()
    return nc


_NC_CACHE = {}
LAST_RESULT = None


def _get_nc(reps=1):
    if reps not in _NC_CACHE:
        _NC_CACHE[reps] = _build_bass(reps)
    return _NC_CACHE[reps]


def _host_prep(x, W0, b0, W1, b1):
    """Build per-core input maps (host-side layout prep, all cheap numpy)."""
    def prep_w(W, nt, odim, dh):
        H = W.shape[1]
        Wp = np.zeros((odim, nt * HS, DC), dtype=np.float32)
        Wp[:, :H, :] = W[:, :, dh * DC:(dh + 1) * DC]
        # (o, hf, d) -> per tile (hf_local, d, o) contiguous; rows 120-127 zero
        tiles = np.zeros((nt, KT, DC * odim), dtype=NPBF16)
        for t in range(nt):
            blk = Wp[:, t * HS:(t + 1) * HS, :]          # (o, 120, DC)
            tiles[t, :HS] = (
                blk.transpose(1, 2, 0).reshape(HS, DC * odim).astype(NPBF16)
            )
        return np.ascontiguousarray(
            tiles.transpose(1, 0, 2).reshape(KT, nt * DC * odim)
        )

    def prep_sel(kdim, nt):
        s = np.zeros((kdim, nt, KT), dtype=NPBF16)
        for t in range(nt):
            for p in range(HS):
                h = 3 * t + p // F
                if h < kdim:
                    s[h, t, p] = 1.0
        return s.reshape(kdim, nt * KT)

    w_half = [
        (prep_w(W0, NT0, O0, dh), prep_w(W1, NT1, O1, dh)) for dh in range(ND)
    ]
    sel0 = prep_sel(F, NT0)
    sel1 = prep_sel(O0, NT1)
    b0h = b0.reshape(O0, 1).astype(np.float32)
    b1h = b1.reshape(O1, 1).astype(np.float32)

    in_maps = []
    for c in range(NCORES):
        bs, dh = c % NB, c // NB
        xc = x[bs * BC:(bs + 1) * BC]                    # (512, 40, 32)
        xtc = np.ascontiguousarray(
            xc[:, :, dh * DC:(dh + 1) * DC].transpose(1, 2, 0).reshape(F, DC * BC)
        ).astype(NPBF16)
        in_maps.append({
            "xt": xtc,
            "w0t": w_half[dh][0],
            "w1t": w_half[dh][1],
            "sel0": sel0,
            "sel1": sel1,
            "b0": b0h,
            "b1": b1h,
        })
    return in_maps


def kernel(x, W0, b0, W1, b1):
    global LAST_RESULT
    x = np.asarray(x, dtype=np.float32)
    W0 = np.asarray(W0, dtype=np.float32)
    W1 = np.asarray(W1, dtype=np.float32)
    b0 = np.asarray(b0, dtype=np.float32)
    b1 = np.asarray(b1, dtype=np.float32)

    nc = _get_nc()
    in_maps = _host_prep(x, W0, b0, W1, b1)
    res = run_bass_kernel_spmd(nc, in_maps, core_ids=list(range(NCORES)))
    LAST_RESULT = res

    out = np.empty((B, F + O0 + O1), dtype=np.float32)
    out[:, :F] = x.sum(axis=-1)
    for bs in range(NB):
        half0 = np.asarray(res.results[bs]["out"])
        half1 = np.asarray(res.results[NB + bs]["out"])
        out[bs * BC:(bs + 1) * BC, F:] = half0 + half1
    return out



# revision 2
# speedup vs baseline: 1.3212x; 1.3212x over previous
"""Trainium2 Bass kernel for the 2-layer CIN (compressed interaction network).

Reference computation (per batch element b, embedding channel d):
  z0[hf=h*40+f]  = x[b,h,d] * x[b,f,d]              (h,f in 0..39)
  y0[o]          = relu(sum_hf W0[o,hf,d] * z0[hf] + b0[o])   -> x1[b,o,d]
  z1[hf=h1*40+f] = x1[b,h1,d] * x[b,f,d]            (h1 in 0..63)
  y1[o]          = relu(sum_hf W1[o,hf,d] * z1[hf] + b1[o])   -> x2[b,o,d]
  out[b] = [sum_d x[b,:,d] | sum_d x1[b,:,d] | sum_d x2[b,:,d]]   (2048, 168)

Sharding: 4-way batch x 2-way embedding-channel split (8 cores); each core
computes partial d-sums for its 512-row shard over its 16 d-channels; the
host adds the two d-halves. Input dtypes preserved (fp32 in/out).

Design (vs the v1 selection-matmul baseline, ~9x fewer stall cycles):
  - Layer 1 is SYMMETRIZED on the host (W0s = W0 + W0^T off-diagonal): only
    the 820 unique (h<=f) products are computed, packed into 9 K-tiles with
    a fixed per-partition f-pattern (resident XF1/XF8) and AP-expressible
    per-tile h-patterns. Layer-1 matmul passes drop 28 -> 18 per d-group.
  - z-tiles are built by one of three production paths, cycled per tile
    (PROD_CYCLE): F = selection-matmul into PSUM + fused DVE multiply
    directly from PSUM (no copy); S = selection-matmul + ScalarE copy +
    Pool multiply; D = partition-replicating DMA (stride-0 APs, spread over
    the SP/Act/Pool queues) + Pool multiply.
  - The 248 K-tiles per rep are emitted as ONE flat software-pipelined
    stream: each tile's production is emitted LOOKAHEAD tiles ahead of its
    consuming matmuls, so the in-order engine queues never head-of-line
    block on cross-engine production chains (real-HW semaphore hops are
    ~1.5us; without lookahead every tile serialized at ~3.3us).
  - relu+bias fused as one (64,1024) ScalarE activation per (group, layer);
    d-sums accumulate in bf16 pair tiles, combined + PE-transposed in the
    epilogue.
"""

import os
from contextlib import ExitStack

import numpy as np
import ml_dtypes

import concourse.bass as bass
import concourse.bacc as bacc
import concourse.tile as tile
from concourse import mybir
from concourse.bass_utils import run_bass_kernel_spmd

BF16 = mybir.dt.bfloat16
FP32 = mybir.dt.float32
NPBF16 = ml_dtypes.bfloat16

B, F, D = 2048, 40, 32
O0, O1 = 64, 64
NCORES = 8
NB = 4                      # batch shards
ND = 2                      # d shards
BC = B // NB                # 512 batch rows per core
DC = D // ND                # 16 embedding channels per core
NT1 = 9                     # L1 K-tiles (symmetric triangular packing)
NT2 = 22                    # L2 K-tiles (3h x 40f packing)
DG = DC // 2                # 8 d-groups (2 d per group)
NCOL = 2 * BC               # 1024 free columns per group (d-major)
NMM = 512                   # max fp32-PSUM matmul free size
W = DC * BC                 # 8192 resident free width

# --- L1 triangular slot maps ----------------------------------------------
# Tiles t=0..7: h(p) = t + HOFF[p] (4 runs of 32 -> ONE replication DMA);
# the f-side pattern F0 is tile-independent (resident XF1). Pads carry W=0.
# Tile 8 mops up pairs {a<=7, b>=32} (role-swapped: h=b, f=a) and
# {32<=f<=h<=39}: h(p) = 32+p//16 (ONE DMA), f8(p) from resident XF8.


def _l1_slotmap():
    hoff = np.zeros(128, dtype=np.int64)
    f0 = np.zeros(128, dtype=np.int64)
    pad = np.zeros(128, dtype=bool)
    for p in range(128):
        hoff[p] = 8 * (p // 32)
        if p < 32:
            f0[p] = p
        elif p < 64:
            f0[p] = p - 24
        elif p < 88:
            f0[p] = p - 48
        elif p < 96:
            f0[p], pad[p] = 32 + (p - 88), True
        elif p < 112:
            f0[p] = 24 + (p - 96)
        else:
            f0[p], pad[p] = 32 + ((p - 112) % 8), True
    p = np.arange(128)
    h8 = 32 + p % 8
    f8 = np.where(p < 64, p // 8, 32 + (p - 64) // 8)
    return hoff, f0, pad, h8, f8


HOFF, F0, PAD0, H8, F8 = _l1_slotmap()

# DMA queue cycle (program order) and Pool multiply slots (tunable)
QUEUE_CYCLE = ["sp", "act", "sp", "act", "sp", "act", "pool"]
POOL_MUL_SLOTS = {2, 5, 8, 11, 13}   # of every 14 multiplies

# Production mode per tile, cycled in emission order (tunable).
#   F = sel-matmul -> PSUM, fused DVE multiply from PSUM
#   S = sel-matmul -> PSUM, ScalarE copy -> SBUF, Pool multiply
#   D = replication DMA -> SBUF, Pool multiply
PROD_CYCLE = "FFDFF"
LOOKAHEAD = 6               # tiles of z-production ahead of consumption


class _QueueSched:
    """Strict program-order round-robin over the 3 DMA queues."""

    def __init__(self, nc):
        self.eng = {"sp": nc.sync, "act": nc.scalar, "pool": nc.gpsimd}
        self.i = 0

    def pick(self):
        q = QUEUE_CYCLE[self.i % len(QUEUE_CYCLE)]
        self.i += 1
        return self.eng[q]


def _build_bass(reps=1):
    nc = bacc.Bacc()
    xt = nc.declare_dram_parameter("xt", [F, W], BF16, isOutput=False)
    w0t = nc.declare_dram_parameter("w0t", [128, NT1 * DC * O0], BF16, isOutput=False)
    w1t = nc.declare_dram_parameter("w1t", [128, NT2 * DC * O1], BF16, isOutput=False)
    sel0 = nc.declare_dram_parameter("sel0", [F, NT1 * 128], BF16, isOutput=False)
    sel1 = nc.declare_dram_parameter("sel1", [O0, NT2 * 128], BF16, isOutput=False)
    b0 = nc.declare_dram_parameter("b0", [O0, 1], FP32, isOutput=False)
    b1 = nc.declare_dram_parameter("b1", [O1, 1], FP32, isOutput=False)
    out = nc.declare_dram_parameter("out", [BC, O0 + O1], FP32, isOutput=True)

    with ExitStack() as ctx:
        tc = ctx.enter_context(tile.TileContext(nc))
        singles = ctx.enter_context(tc.tile_pool(name="singles", bufs=1))
        y_ps = ctx.enter_context(tc.tile_pool(name="y_ps", bufs=2, space="PSUM"))
        xh_ps = ctx.enter_context(tc.tile_pool(name="xh_ps", bufs=2, space="PSUM"))
        xh_sb = ctx.enter_context(tc.tile_pool(name="xh_sb", bufs=4))
        z_sb = ctx.enter_context(tc.tile_pool(name="z_sb", bufs=6))
        x2_sb = ctx.enter_context(tc.tile_pool(name="x2_sb", bufs=2))
        o_sb = ctx.enter_context(tc.tile_pool(name="o_sb", bufs=2))

        # ---- resident tensors ----
        xt_sb = singles.tile([F, W], BF16)
        sel0s = singles.tile([F, NT1, 128], BF16)
        sel1s = singles.tile([O0, NT2, 128], BF16)
        xf1 = singles.tile([128, W], BF16)
        xf2 = singles.tile([128, W], BF16)
        xf8 = singles.tile([128, W], BF16)
        w0s = singles.tile([128, NT1, DC * O0], BF16)
        w1s = singles.tile([128, NT2, DC * O1], BF16)
        b0s = singles.tile([O0, 1], FP32)
        b1s = singles.tile([O1, 1], FP32)
        x1t = singles.tile([O0, W], BF16)
        acc1p = singles.tile([O0, NCOL], BF16)
        acc2p = singles.tile([O1, NCOL], BF16)
        acc1f = singles.tile([O0, BC], FP32)
        acc2f = singles.tile([O1, BC], FP32)

        from concourse.masks import make_identity
        ident = singles.tile([128, 128], FP32)
        make_identity(nc, ident)

        xt_ap = xt[:]

        def dram_rep(offset_elems, ap):
            return bass.AP(tensor=xt_ap.tensor, offset=xt_ap.offset + offset_elems,
                           ap=ap)

        def load_inputs():
            lq = _QueueSched(nc)
            lq.pick().dma_start(out=xt_sb, in_=xt[:])
            lq.pick().dma_start(out=sel0s, in_=sel0[:])
            lq.pick().dma_start(out=sel1s, in_=sel1[:])
            # XF2: partition p = rep*40 + f holds xT row f (pad: rows 0..7)
            lq.pick().dma_start(
                out=xf2[0:3 * F, :], in_=dram_rep(0, [[0, 3], [W, F], [1, W]])
            )
            lq.pick().dma_start(
                out=xf2[3 * F:128, :], in_=dram_rep(0, [[W, 128 - 3 * F], [1, W]])
            )
            # XF1: fixed f0 pattern, contiguous row-runs (+ one 2x8 run)
            runs = [(0, 0, 32), (32, 8, 32), (64, 16, 24), (88, 32, 8),
                    (96, 24, 16)]
            for pstart, row0, n in runs:
                lq.pick().dma_start(
                    out=xf1[pstart:pstart + n, :],
                    in_=dram_rep(row0 * W, [[W, n], [1, W]]),
                )
            lq.pick().dma_start(
                out=xf1[112:128, :],
                in_=dram_rep(32 * W, [[0, 2], [W, 8], [1, W]]),
            )
            # XF8: f = p//8 on [0:64), f = 32+(p-64)//8 on [64:128)
            for pstart, row0 in ((0, 0), (64, 32)):
                lq.pick().dma_start(
                    out=xf8[pstart:pstart + 64, :],
                    in_=dram_rep(row0 * W, [[W, 8], [0, 8], [1, W]]),
                )
            lq.pick().dma_start(out=w0s, in_=w0t[:])
            lq.pick().dma_start(out=w1s, in_=w1t[:])
            lq.pick().dma_start(out=b0s, in_=b0[:])
            lq.pick().dma_start(out=b1s, in_=b1[:])

        load_inputs()

        x1t_ap = x1t[:]

        for rep in range(reps):
            qs = _QueueSched(nc)
            prod_i = 0

            def produce(name, t, g, kp, dma_src, sels, rhs_sb, krows, xf):
                """Build z = XH * XF for one K-tile; returns the z tile.

                Production mode cycles through PROD_CYCLE: F = sel-matmul +
                fused DVE multiply from PSUM; S = sel-matmul + ScalarE copy
                + Pool multiply; D = replication DMA + Pool multiply.
                """
                nonlocal prod_i
                mode = PROD_CYCLE[prod_i % len(PROD_CYCLE)]
                prod_i += 1
                col0 = g * NCOL
                z = z_sb.tile([128, NCOL], BF16, tag="z", name=f"z{name}")
                xfs = xf[0:kp, col0:col0 + NCOL]
                if mode == "D":
                    xh = xh_sb.tile([128, NCOL], BF16, tag="xh",
                                    name=f"xh{name}")
                    qs.pick().dma_start(out=xh[0:kp, :], in_=dma_src)
                    nc.gpsimd.tensor_mul(z[0:kp, :], xh[0:kp, :], xfs)
                    return z
                xp = xh_ps.tile([128, NCOL], FP32, tag="xp", name=f"xp{name}")
                for i in range(2):
                    nc.tensor.matmul(
                        xp[:, i * NMM:(i + 1) * NMM],
                        lhsT=sels[:, t, :],
                        rhs=rhs_sb[0:krows, col0 + i * NMM:col0 + (i + 1) * NMM],
                        start=True,
                        stop=True,
                    )
                if mode == "F":
                    nc.vector.tensor_mul(z[0:kp, :], xp[0:kp, :], xfs)
                else:
                    xh = xh_sb.tile([128, NCOL], BF16, tag="xh",
                                    name=f"xh{name}")
                    nc.scalar.copy(out=xh[0:kp, :], in_=xp[0:kp, :])
                    nc.gpsimd.tensor_mul(z[0:kp, :], xh[0:kp, :], xfs)
                return z

            nc.vector.memset(acc1p, 0.0)
            nc.vector.memset(acc2p, 0.0)

            # ---- flat software-pipelined tile stream ----
            # Tile order interleaves groups (L1(0), L1(1), L2(0), L1(2),
            # L2(1), ..., L2(7)); z-productions are emitted LOOKAHEAD tiles
            # ahead of their consuming matmuls so the in-order engine queues
            # never stall on the cross-engine production chains.
            def l1_spec(g, t):
                col0 = g * NCOL
                if t < 8:
                    src = dram_rep(t * W + col0,
                                   [[8 * W, 4], [0, 32], [1, NCOL]])
                    xf = xf1
                else:
                    src = dram_rep(32 * W + col0,
                                   [[0, 16], [W, 8], [1, NCOL]])
                    xf = xf8
                return dict(name=f"1_{rep}_{g}_{t}", t=t, g=g, kp=128,
                            src=src, sels=sel0s, rhs=xt_sb, krows=F, xf=xf,
                            ws=w0s, wk=128, blk=("L1", g), last=(t == NT1 - 1))

            def l2_spec(g, t):
                col0 = g * NCOL
                nh = 3 if t < NT2 - 1 else 1
                kp = nh * F
                src = bass.AP(tensor=x1t_ap.tensor,
                              offset=x1t_ap.offset + 3 * t * W + col0,
                              ap=[[W, nh], [0, F], [1, NCOL]])
                return dict(name=f"2_{rep}_{g}_{t}", t=t, g=g, kp=kp,
                            src=src, sels=sel1s, rhs=x1t, krows=O0, xf=xf2,
                            ws=w1s, wk=kp, blk=("L2", g), last=(t == NT2 - 1))

            seq = [l1_spec(0, t) for t in range(NT1)]
            seq += [l1_spec(1, t) for t in range(NT1)]
            for g in range(DG):
                seq += [l2_spec(g, t) for t in range(NT2)]
                if g + 2 < DG:
                    seq += [l1_spec(g + 2, t) for t in range(NT1)]

            yps = {}

            def finish_block(blk):
                kind, g = blk
                col0 = g * NCOL
                yp = yps.pop(blk)
                if kind == "L1":
                    nc.scalar.activation(
                        out=x1t[:, col0:col0 + NCOL], in_=yp,
                        func=mybir.ActivationFunctionType.Relu,
                        bias=b0s, scale=1.0,
                    )
                    nc.vector.tensor_add(acc1p, acc1p,
                                         x1t[:, col0:col0 + NCOL])
                else:
                    x2 = x2_sb.tile([O1, NCOL], BF16, tag="x2",
                                    name=f"x2_{rep}_{g}")
                    nc.scalar.activation(
                        out=x2, in_=yp,
                        func=mybir.ActivationFunctionType.Relu,
                        bias=b1s, scale=1.0,
                    )
                    nc.vector.tensor_add(acc2p, acc2p, x2)

            def consume(s, z):
                blk = s["blk"]
                if blk not in yps:
                    yps[blk] = y_ps.tile([O0, NCOL], FP32, tag="y",
                                         name=f"y{blk[0]}_{rep}_{blk[1]}")
                yp = yps[blk]
                kp = s["kp"]
                for i in range(2):
                    nc.tensor.matmul(
                        yp[:, i * NMM:(i + 1) * NMM],
                        lhsT=s["ws"][0:s["wk"], s["t"],
                                     (2 * s["g"] + i) * O0:
                                     (2 * s["g"] + i + 1) * O0],
                        rhs=z[0:kp, i * NMM:(i + 1) * NMM],
                        start=(s["t"] == 0),
                        stop=s["last"],
                    )
                if s["last"]:
                    finish_block(blk)

            zq = []
            for s in seq:
                z = produce(s["name"], s["t"], s["g"], s["kp"], s["src"],
                            s["sels"], s["rhs"], s["krows"], s["xf"])
                zq.append((s, z))
                if len(zq) > LOOKAHEAD:
                    sc, zc = zq.pop(0)
                    consume(sc, zc)
            for sc, zc in zq:
                consume(sc, zc)

            # ---- epilogue: combine bf16 d-half pairs, transpose, store ----
            nc.vector.tensor_add(acc1f, acc1p[:, 0:BC], acc1p[:, BC:NCOL])
            nc.vector.tensor_add(acc2f, acc2p[:, 0:BC], acc2p[:, BC:NCOL])
            for bh in range(BC // 128):
                outT = o_sb.tile([128, O0 + O1], FP32, tag="outT",
                                 name=f"outT_{rep}_{bh}")
                for acc, off in ((acc1f, 0), (acc2f, O0)):
                    pt = xh_ps.tile([128, NCOL], FP32, tag="xp",
                                    name=f"pt_{rep}_{bh}_{off}")
                    nc.tensor.transpose(
                        pt[:, 0:64], acc[:, bh * 128:(bh + 1) * 128],
                        ident[0:64, 0:64]
                    )
                    nc.vector.tensor_copy(out=outT[:, off:off + 64],
                                          in_=pt[:, 0:64])
                nc.sync.dma_start(
                    out=out[bh * 128:(bh + 1) * 128, :], in_=outT
                )

    nc.compile()
    return nc


_NC_CACHE = {}
LAST_RESULT = None


def _get_nc(reps=1):
    if reps not in _NC_CACHE:
        _NC_CACHE[reps] = _build_bass(reps)
    return _NC_CACHE[reps]


def _host_prep(x, W0, b0, W1, b1):
    """Build per-core input maps (host-side layout prep, cheap numpy)."""
    # ---- symmetrized L1 weights -> (128, NT1, DC, O0) tiles ----
    W0r = W0.reshape(O0, F, F, D)                       # (o, h, f, d)
    W0sym = W0r + W0r.transpose(0, 2, 1, 3)             # symmetric, h!=f summed
    di = np.arange(F)
    W0sym[:, di, di, :] = W0r[:, di, di, :]             # diagonal not doubled

    W1r = W1.reshape(O1, O0, F, D)                      # (o, h1, f, d)

    def prep_w0(dh):
        Wd = W0sym[:, :, :, dh * DC:(dh + 1) * DC]      # (o, h, f, DC)
        tiles = np.zeros((128, NT1, DC, O0), dtype=NPBF16)
        for t in range(NT1):
            if t < 8:
                h, f, pad = t + HOFF, F0, PAD0
            else:
                h, f, pad = H8, F8, np.zeros(128, dtype=bool)
            lo, hi = np.minimum(h, f), np.maximum(h, f)
            valid = (~pad) & (h >= f if t == 8 else h <= f)
            blk = Wd[:, lo[valid], hi[valid], :]        # (o, nv, DC)
            tiles[valid, t] = blk.transpose(1, 2, 0).astype(NPBF16)
        return np.ascontiguousarray(tiles.reshape(128, NT1 * DC * O0))

    def prep_w1(dh):
        Wd = W1r[:, :, :, dh * DC:(dh + 1) * DC]        # (o, h1, f, DC)
        tiles = np.zeros((128, NT2, DC, O1), dtype=NPBF16)
        p = np.arange(120)
        for t in range(NT2):
            h = 3 * t + p // F
            f = p % F
            valid = h < O0
            blk = Wd[:, h[valid], f[valid], :]          # (o, nv, DC)
            tiles[p[valid], t] = blk.transpose(1, 2, 0).astype(NPBF16)
        return np.ascontiguousarray(tiles.reshape(128, NT2 * DC * O1))

    w_half = [(prep_w0(dh), prep_w1(dh)) for dh in range(ND)]
    b0h = b0.reshape(O0, 1).astype(np.float32)
    b1h = b1.reshape(O1, 1).astype(np.float32)

    # selection matrices: sel[k, t*128+p] = 1 iff h_t(p) == k
    sel0h = np.zeros((F, NT1, 128), dtype=NPBF16)
    for t in range(NT1):
        h = (t + HOFF) if t < 8 else H8
        sel0h[h, t, np.arange(128)] = 1.0
    sel1h = np.zeros((O0, NT2, 128), dtype=NPBF16)
    p120 = np.arange(120)
    for t in range(NT2):
        h = 3 * t + p120 // F
        m = h < O0
        sel1h[h[m], t, p120[m]] = 1.0
    sel0h = np.ascontiguousarray(sel0h.reshape(F, NT1 * 128))
    sel1h = np.ascontiguousarray(sel1h.reshape(O0, NT2 * 128))

    in_maps = []
    for c in range(NCORES):
        bs, dh = c % NB, c // NB
        xc = x[bs * BC:(bs + 1) * BC]                   # (512, 40, 32)
        xtc = np.ascontiguousarray(
            xc[:, :, dh * DC:(dh + 1) * DC].transpose(1, 2, 0).reshape(F, W)
        ).astype(NPBF16)
        in_maps.append({
            "xt": xtc,
            "w0t": w_half[dh][0],
            "w1t": w_half[dh][1],
            "sel0": sel0h,
            "sel1": sel1h,
            "b0": b0h,
            "b1": b1h,
        })
    return in_maps


def kernel(x, W0, b0, W1, b1):
    global LAST_RESULT
    x = np.asarray(x, dtype=np.float32)
    W0 = np.asarray(W0, dtype=np.float32)
    W1 = np.asarray(W1, dtype=np.float32)
    b0 = np.asarray(b0, dtype=np.float32)
    b1 = np.asarray(b1, dtype=np.float32)

    nc = _get_nc()
    in_maps = _host_prep(x, W0, b0, W1, b1)
    res = run_bass_kernel_spmd(nc, in_maps, core_ids=list(range(NCORES)))
    LAST_RESULT = res

    out = np.empty((B, F + O0 + O1), dtype=np.float32)
    out[:, :F] = x.sum(axis=-1)
    for bs in range(NB):
        half0 = np.asarray(res.results[bs]["out"])
        half1 = np.asarray(res.results[NB + bs]["out"])
        out[bs * BC:(bs + 1) * BC, F:] = half0 + half1
    return out


# revision 3
# speedup vs baseline: 1.7355x; 1.3136x over previous
"""Trainium2 Bass kernel for the 2-layer CIN (compressed interaction network).

Reference computation (per batch element b, embedding channel d):
  z0[hf=h*40+f]  = x[b,h,d] * x[b,f,d]              (h,f in 0..39)
  y0[o]          = relu(sum_hf W0[o,hf,d] * z0[hf] + b0[o])   -> x1[b,o,d]
  z1[hf=h1*40+f] = x1[b,h1,d] * x[b,f,d]            (h1 in 0..63)
  y1[o]          = relu(sum_hf W1[o,hf,d] * z1[hf] + b1[o])   -> x2[b,o,d]
  out[b] = [sum_d x[b,:,d] | sum_d x1[b,:,d] | sum_d x2[b,:,d]]   (2048, 168)

Sharding: 4-way batch x 2-way embedding-channel split (8 cores); each core
computes partial d-sums for its 512-row shard over its 16 d-channels; the
host adds the two d-halves. Input dtypes preserved (fp32 in/out).

Design (vs the v1 selection-matmul baseline, ~9x fewer stall cycles):
  - Layer 1 is SYMMETRIZED on the host (W0s = W0 + W0^T off-diagonal): only
    the 820 unique (h<=f) products are computed, packed into 9 K-tiles with
    a fixed per-partition f-pattern (resident XF1/XF8) and AP-expressible
    per-tile h-patterns. Layer-1 matmul passes drop 28 -> 18 per d-group.
  - z-tiles are built by one of three production paths, cycled per tile
    (PROD_CYCLE): F = selection-matmul into PSUM + fused DVE multiply
    directly from PSUM (no copy); S = selection-matmul + ScalarE copy +
    Pool multiply; D = partition-replicating DMA (stride-0 APs, spread over
    the SP/Act/Pool queues) + Pool multiply. Default is 80%% F / 20%% S: a
    DMA-free rep body is both fastest and robust to DMA-engine state
    (replication DMAs measured 1.9-20us/tile across device sessions).
  - The 248 K-tiles per rep are emitted as ONE flat software-pipelined
    stream: each tile's production is emitted LOOKAHEAD tiles ahead of its
    consuming matmuls, so the in-order engine queues never head-of-line
    block on cross-engine production chains (real-HW semaphore hops are
    ~1.5us; without lookahead every tile serialized at ~3.3us).
  - relu+bias fused as one (64,1024) ScalarE activation per (group, layer);
    d-sums accumulate in bf16 pair tiles, combined + PE-transposed in the
    epilogue.
"""

import os
from contextlib import ExitStack

import numpy as np
import ml_dtypes

import concourse.bass as bass
import concourse.bacc as bacc
import concourse.tile as tile
from concourse import mybir
from concourse.bass_utils import run_bass_kernel_spmd

BF16 = mybir.dt.bfloat16
FP32 = mybir.dt.float32
NPBF16 = ml_dtypes.bfloat16

B, F, D = 2048, 40, 32
O0, O1 = 64, 64
NCORES = 8
NB = 4                      # batch shards
ND = 2                      # d shards
BC = B // NB                # 512 batch rows per core
DC = D // ND                # 16 embedding channels per core
NT1 = 9                     # L1 K-tiles (symmetric triangular packing)
NT2 = 22                    # L2 K-tiles (3h x 40f packing)
DG = DC // 2                # 8 d-groups (2 d per group)
NCOL = 2 * BC               # 1024 free columns per group (d-major)
NMM = 512                   # max fp32-PSUM matmul free size
W = DC * BC                 # 8192 resident free width

# --- L1 triangular slot maps ----------------------------------------------
# Tiles t=0..7: h(p) = t + HOFF[p] (4 runs of 32 -> ONE replication DMA);
# the f-side pattern F0 is tile-independent (resident XF1). Pads carry W=0.
# Tile 8 mops up pairs {a<=7, b>=32} (role-swapped: h=b, f=a) and
# {32<=f<=h<=39}: h(p) = 32+p//16 (ONE DMA), f8(p) from resident XF8.


def _l1_slotmap():
    hoff = np.zeros(128, dtype=np.int64)
    f0 = np.zeros(128, dtype=np.int64)
    pad = np.zeros(128, dtype=bool)
    for p in range(128):
        hoff[p] = 8 * (p // 32)
        if p < 32:
            f0[p] = p
        elif p < 64:
            f0[p] = p - 24
        elif p < 88:
            f0[p] = p - 48
        elif p < 96:
            f0[p], pad[p] = 32 + (p - 88), True
        elif p < 112:
            f0[p] = 24 + (p - 96)
        else:
            f0[p], pad[p] = 32 + ((p - 112) % 8), True
    p = np.arange(128)
    h8 = 32 + p % 8
    f8 = np.where(p < 64, p // 8, 32 + (p - 64) // 8)
    return hoff, f0, pad, h8, f8


HOFF, F0, PAD0, H8, F8 = _l1_slotmap()

# DMA queue cycle (program order) and Pool multiply slots (tunable)
QUEUE_CYCLE = ["sp", "act", "sp", "act", "sp", "act", "pool"]
POOL_MUL_SLOTS = {2, 5, 8, 11, 13}   # of every 14 multiplies

# Production mode per tile, cycled in emission order (tunable).
#   F = sel-matmul -> PSUM, fused DVE multiply from PSUM
#   S = sel-matmul -> PSUM, ScalarE copy -> SBUF, Pool multiply
#   D = replication DMA -> SBUF, Pool multiply
PROD_CYCLE = "FFFSF"
LOOKAHEAD = 6               # tiles of z-production ahead of consumption


class _QueueSched:
    """Strict program-order round-robin over the 3 DMA queues."""

    def __init__(self, nc):
        self.eng = {"sp": nc.sync, "act": nc.scalar, "pool": nc.gpsimd}
        self.i = 0

    def pick(self):
        q = QUEUE_CYCLE[self.i % len(QUEUE_CYCLE)]
        self.i += 1
        return self.eng[q]


def _build_bass(reps=1):
    nc = bacc.Bacc()
    xt = nc.declare_dram_parameter("xt", [F, W], BF16, isOutput=False)
    w0t = nc.declare_dram_parameter("w0t", [128, NT1 * DC * O0], BF16, isOutput=False)
    w1t = nc.declare_dram_parameter("w1t", [128, NT2 * DC * O1], BF16, isOutput=False)
    sel0 = nc.declare_dram_parameter("sel0", [F, NT1 * 128], BF16, isOutput=False)
    sel1 = nc.declare_dram_parameter("sel1", [O0, NT2 * 128], BF16, isOutput=False)
    b0 = nc.declare_dram_parameter("b0", [O0, 1], FP32, isOutput=False)
    b1 = nc.declare_dram_parameter("b1", [O1, 1], FP32, isOutput=False)
    out = nc.declare_dram_parameter("out", [BC, O0 + O1], FP32, isOutput=True)

    with ExitStack() as ctx:
        tc = ctx.enter_context(tile.TileContext(nc))
        singles = ctx.enter_context(tc.tile_pool(name="singles", bufs=1))
        y_ps = ctx.enter_context(tc.tile_pool(name="y_ps", bufs=2, space="PSUM"))
        xh_ps = ctx.enter_context(tc.tile_pool(name="xh_ps", bufs=2, space="PSUM"))
        xh_sb = ctx.enter_context(tc.tile_pool(name="xh_sb", bufs=4))
        z_sb = ctx.enter_context(tc.tile_pool(name="z_sb", bufs=6))
        x2_sb = ctx.enter_context(tc.tile_pool(name="x2_sb", bufs=2))
        o_sb = ctx.enter_context(tc.tile_pool(name="o_sb", bufs=2))

        # ---- resident tensors ----
        xt_sb = singles.tile([F, W], BF16)
        sel0s = singles.tile([F, NT1, 128], BF16)
        sel1s = singles.tile([O0, NT2, 128], BF16)
        xf1 = singles.tile([128, W], BF16)
        xf2 = singles.tile([128, W], BF16)
        xf8 = singles.tile([128, W], BF16)
        w0s = singles.tile([128, NT1, DC * O0], BF16)
        w1s = singles.tile([128, NT2, DC * O1], BF16)
        b0s = singles.tile([O0, 1], FP32)
        b1s = singles.tile([O1, 1], FP32)
        x1t = singles.tile([O0, W], BF16)
        acc1p = singles.tile([O0, NCOL], BF16)
        acc2p = singles.tile([O1, NCOL], BF16)
        acc1f = singles.tile([O0, BC], FP32)
        acc2f = singles.tile([O1, BC], FP32)

        from concourse.masks import make_identity
        ident = singles.tile([128, 128], FP32)
        make_identity(nc, ident)

        xt_ap = xt[:]

        def dram_rep(offset_elems, ap):
            return bass.AP(tensor=xt_ap.tensor, offset=xt_ap.offset + offset_elems,
                           ap=ap)

        def load_inputs():
            lq = _QueueSched(nc)
            lq.pick().dma_start(out=xt_sb, in_=xt[:])
            lq.pick().dma_start(out=sel0s, in_=sel0[:])
            lq.pick().dma_start(out=sel1s, in_=sel1[:])
            # XF2: partition p = rep*40 + f holds xT row f (pad: rows 0..7)
            lq.pick().dma_start(
                out=xf2[0:3 * F, :], in_=dram_rep(0, [[0, 3], [W, F], [1, W]])
            )
            lq.pick().dma_start(
                out=xf2[3 * F:128, :], in_=dram_rep(0, [[W, 128 - 3 * F], [1, W]])
            )
            # XF1: fixed f0 pattern, contiguous row-runs (+ one 2x8 run)
            runs = [(0, 0, 32), (32, 8, 32), (64, 16, 24), (88, 32, 8),
                    (96, 24, 16)]
            for pstart, row0, n in runs:
                lq.pick().dma_start(
                    out=xf1[pstart:pstart + n, :],
                    in_=dram_rep(row0 * W, [[W, n], [1, W]]),
                )
            lq.pick().dma_start(
                out=xf1[112:128, :],
                in_=dram_rep(32 * W, [[0, 2], [W, 8], [1, W]]),
            )
            # XF8: f = p//8 on [0:64), f = 32+(p-64)//8 on [64:128)
            for pstart, row0 in ((0, 0), (64, 32)):
                lq.pick().dma_start(
                    out=xf8[pstart:pstart + 64, :],
                    in_=dram_rep(row0 * W, [[W, 8], [0, 8], [1, W]]),
                )
            lq.pick().dma_start(out=w0s, in_=w0t[:])
            lq.pick().dma_start(out=w1s, in_=w1t[:])
            lq.pick().dma_start(out=b0s, in_=b0[:])
            lq.pick().dma_start(out=b1s, in_=b1[:])

        load_inputs()

        x1t_ap = x1t[:]

        for rep in range(reps):
            qs = _QueueSched(nc)
            prod_i = 0

            def produce(name, t, g, kp, dma_src, sels, rhs_sb, krows, xf):
                """Build z = XH * XF for one K-tile; returns the z tile.

                Production mode cycles through PROD_CYCLE: F = sel-matmul +
                fused DVE multiply from PSUM; S = sel-matmul + ScalarE copy
                + Pool multiply; D = replication DMA + Pool multiply.
                """
                nonlocal prod_i
                mode = PROD_CYCLE[prod_i % len(PROD_CYCLE)]
                prod_i += 1
                col0 = g * NCOL
                z = z_sb.tile([128, NCOL], BF16, tag="z", name=f"z{name}")
                xfs = xf[0:kp, col0:col0 + NCOL]
                if mode == "D":
                    xh = xh_sb.tile([128, NCOL], BF16, tag="xh",
                                    name=f"xh{name}")
                    qs.pick().dma_start(out=xh[0:kp, :], in_=dma_src)
                    nc.gpsimd.tensor_mul(z[0:kp, :], xh[0:kp, :], xfs)
                    return z
                xp = xh_ps.tile([128, NCOL], FP32, tag="xp", name=f"xp{name}")
                for i in range(2):
                    nc.tensor.matmul(
                        xp[:, i * NMM:(i + 1) * NMM],
                        lhsT=sels[:, t, :],
                        rhs=rhs_sb[0:krows, col0 + i * NMM:col0 + (i + 1) * NMM],
                        start=True,
                        stop=True,
                    )
                if mode == "F":
                    nc.vector.tensor_mul(z[0:kp, :], xp[0:kp, :], xfs)
                else:
                    xh = xh_sb.tile([128, NCOL], BF16, tag="xh",
                                    name=f"xh{name}")
                    nc.scalar.copy(out=xh[0:kp, :], in_=xp[0:kp, :])
                    nc.gpsimd.tensor_mul(z[0:kp, :], xh[0:kp, :], xfs)
                return z

            nc.vector.memset(acc1p, 0.0)
            nc.vector.memset(acc2p, 0.0)

            # ---- flat software-pipelined tile stream ----
            # Tile order interleaves groups (L1(0), L1(1), L2(0), L1(2),
            # L2(1), ..., L2(7)); z-productions are emitted LOOKAHEAD tiles
            # ahead of their consuming matmuls so the in-order engine queues
            # never stall on the cross-engine production chains.
            def l1_spec(g, t):
                col0 = g * NCOL
                if t < 8:
                    src = dram_rep(t * W + col0,
                                   [[8 * W, 4], [0, 32], [1, NCOL]])
                    xf = xf1
                else:
                    src = dram_rep(32 * W + col0,
                                   [[0, 16], [W, 8], [1, NCOL]])
                    xf = xf8
                return dict(name=f"1_{rep}_{g}_{t}", t=t, g=g, kp=128,
                            src=src, sels=sel0s, rhs=xt_sb, krows=F, xf=xf,
                            ws=w0s, wk=128, blk=("L1", g), last=(t == NT1 - 1))

            def l2_spec(g, t):
                col0 = g * NCOL
                nh = 3 if t < NT2 - 1 else 1
                kp = nh * F
                src = bass.AP(tensor=x1t_ap.tensor,
                              offset=x1t_ap.offset + 3 * t * W + col0,
                              ap=[[W, nh], [0, F], [1, NCOL]])
                return dict(name=f"2_{rep}_{g}_{t}", t=t, g=g, kp=kp,
                            src=src, sels=sel1s, rhs=x1t, krows=O0, xf=xf2,
                            ws=w1s, wk=kp, blk=("L2", g), last=(t == NT2 - 1))

            seq = [l1_spec(0, t) for t in range(NT1)]
            seq += [l1_spec(1, t) for t in range(NT1)]
            for g in range(DG):
                seq += [l2_spec(g, t) for t in range(NT2)]
                if g + 2 < DG:
                    seq += [l1_spec(g + 2, t) for t in range(NT1)]

            yps = {}

            def finish_block(blk):
                kind, g = blk
                col0 = g * NCOL
                yp = yps.pop(blk)
                if kind == "L1":
                    nc.scalar.activation(
                        out=x1t[:, col0:col0 + NCOL], in_=yp,
                        func=mybir.ActivationFunctionType.Relu,
                        bias=b0s, scale=1.0,
                    )
                    nc.vector.tensor_add(acc1p, acc1p,
                                         x1t[:, col0:col0 + NCOL])
                else:
                    x2 = x2_sb.tile([O1, NCOL], BF16, tag="x2",
                                    name=f"x2_{rep}_{g}")
                    nc.scalar.activation(
                        out=x2, in_=yp,
                        func=mybir.ActivationFunctionType.Relu,
                        bias=b1s, scale=1.0,
                    )
                    nc.vector.tensor_add(acc2p, acc2p, x2)

            def consume(s, z):
                blk = s["blk"]
                if blk not in yps:
                    yps[blk] = y_ps.tile([O0, NCOL], FP32, tag="y",
                                         name=f"y{blk[0]}_{rep}_{blk[1]}")
                yp = yps[blk]
                kp = s["kp"]
                for i in range(2):
                    nc.tensor.matmul(
                        yp[:, i * NMM:(i + 1) * NMM],
                        lhsT=s["ws"][0:s["wk"], s["t"],
                                     (2 * s["g"] + i) * O0:
                                     (2 * s["g"] + i + 1) * O0],
                        rhs=z[0:kp, i * NMM:(i + 1) * NMM],
                        start=(s["t"] == 0),
                        stop=s["last"],
                    )
                if s["last"]:
                    finish_block(blk)

            zq = []
            for s in seq:
                z = produce(s["name"], s["t"], s["g"], s["kp"], s["src"],
                            s["sels"], s["rhs"], s["krows"], s["xf"])
                zq.append((s, z))
                if len(zq) > LOOKAHEAD:
                    sc, zc = zq.pop(0)
                    consume(sc, zc)
            for sc, zc in zq:
                consume(sc, zc)

            # ---- epilogue: combine bf16 d-half pairs, transpose, store ----
            nc.vector.tensor_add(acc1f, acc1p[:, 0:BC], acc1p[:, BC:NCOL])
            nc.vector.tensor_add(acc2f, acc2p[:, 0:BC], acc2p[:, BC:NCOL])
            for bh in range(BC // 128):
                outT = o_sb.tile([128, O0 + O1], FP32, tag="outT",
                                 name=f"outT_{rep}_{bh}")
                for acc, off in ((acc1f, 0), (acc2f, O0)):
                    pt = xh_ps.tile([128, NCOL], FP32, tag="xp",
                                    name=f"pt_{rep}_{bh}_{off}")
                    nc.tensor.transpose(
                        pt[:, 0:64], acc[:, bh * 128:(bh + 1) * 128],
                        ident[0:64, 0:64]
                    )
                    nc.vector.tensor_copy(out=outT[:, off:off + 64],
                                          in_=pt[:, 0:64])
                nc.sync.dma_start(
                    out=out[bh * 128:(bh + 1) * 128, :], in_=outT
                )

    nc.compile()
    return nc


_NC_CACHE = {}
LAST_RESULT = None


def _get_nc(reps=1):
    if reps not in _NC_CACHE:
        _NC_CACHE[reps] = _build_bass(reps)
    return _NC_CACHE[reps]


def _host_prep(x, W0, b0, W1, b1):
    """Build per-core input maps (host-side layout prep, cheap numpy)."""
    # ---- symmetrized L1 weights -> (128, NT1, DC, O0) tiles ----
    W0r = W0.reshape(O0, F, F, D)                       # (o, h, f, d)
    W0sym = W0r + W0r.transpose(0, 2, 1, 3)             # symmetric, h!=f summed
    di = np.arange(F)
    W0sym[:, di, di, :] = W0r[:, di, di, :]             # diagonal not doubled

    W1r = W1.reshape(O1, O0, F, D)                      # (o, h1, f, d)

    def prep_w0(dh):
        Wd = W0sym[:, :, :, dh * DC:(dh + 1) * DC]      # (o, h, f, DC)
        tiles = np.zeros((128, NT1, DC, O0), dtype=NPBF16)
        for t in range(NT1):
            if t < 8:
                h, f, pad = t + HOFF, F0, PAD0
            else:
                h, f, pad = H8, F8, np.zeros(128, dtype=bool)
            lo, hi = np.minimum(h, f), np.maximum(h, f)
            valid = (~pad) & (h >= f if t == 8 else h <= f)
            blk = Wd[:, lo[valid], hi[valid], :]        # (o, nv, DC)
            tiles[valid, t] = blk.transpose(1, 2, 0).astype(NPBF16)
        return np.ascontiguousarray(tiles.reshape(128, NT1 * DC * O0))

    def prep_w1(dh):
        Wd = W1r[:, :, :, dh * DC:(dh + 1) * DC]        # (o, h1, f, DC)
        tiles = np.zeros((128, NT2, DC, O1), dtype=NPBF16)
        p = np.arange(120)
        for t in range(NT2):
            h = 3 * t + p // F
            f = p % F
            valid = h < O0
            blk = Wd[:, h[valid], f[valid], :]          # (o, nv, DC)
            tiles[p[valid], t] = blk.transpose(1, 2, 0).astype(NPBF16)
        return np.ascontiguousarray(tiles.reshape(128, NT2 * DC * O1))

    w_half = [(prep_w0(dh), prep_w1(dh)) for dh in range(ND)]
    b0h = b0.reshape(O0, 1).astype(np.float32)
    b1h = b1.reshape(O1, 1).astype(np.float32)

    # selection matrices: sel[k, t*128+p] = 1 iff h_t(p) == k
    sel0h = np.zeros((F, NT1, 128), dtype=NPBF16)
    for t in range(NT1):
        h = (t + HOFF) if t < 8 else H8
        sel0h[h, t, np.arange(128)] = 1.0
    sel1h = np.zeros((O0, NT2, 128), dtype=NPBF16)
    p120 = np.arange(120)
    for t in range(NT2):
        h = 3 * t + p120 // F
        m = h < O0
        sel1h[h[m], t, p120[m]] = 1.0
    sel0h = np.ascontiguousarray(sel0h.reshape(F, NT1 * 128))
    sel1h = np.ascontiguousarray(sel1h.reshape(O0, NT2 * 128))

    in_maps = []
    for c in range(NCORES):
        bs, dh = c % NB, c // NB
        xc = x[bs * BC:(bs + 1) * BC]                   # (512, 40, 32)
        xtc = np.ascontiguousarray(
            xc[:, :, dh * DC:(dh + 1) * DC].transpose(1, 2, 0).reshape(F, W)
        ).astype(NPBF16)
        in_maps.append({
            "xt": xtc,
            "w0t": w_half[dh][0],
            "w1t": w_half[dh][1],
            "sel0": sel0h,
            "sel1": sel1h,
            "b0": b0h,
            "b1": b1h,
        })
    return in_maps


def kernel(x, W0, b0, W1, b1):
    global LAST_RESULT
    x = np.asarray(x, dtype=np.float32)
    W0 = np.asarray(W0, dtype=np.float32)
    W1 = np.asarray(W1, dtype=np.float32)
    b0 = np.asarray(b0, dtype=np.float32)
    b1 = np.asarray(b1, dtype=np.float32)

    nc = _get_nc()
    in_maps = _host_prep(x, W0, b0, W1, b1)
    res = run_bass_kernel_spmd(nc, in_maps, core_ids=list(range(NCORES)))
    LAST_RESULT = res

    out = np.empty((B, F + O0 + O1), dtype=np.float32)
    out[:, :F] = x.sum(axis=-1)
    for bs in range(NB):
        half0 = np.asarray(res.results[bs]["out"])
        half1 = np.asarray(res.results[NB + bs]["out"])
        out[bs * BC:(bs + 1) * BC, F:] = half0 + half1
    return out
